# revision 8
# baseline (speedup 1.0000x reference)
"""Trainium2 Bass kernel for nn_MetaLayer (GNN message passing MetaLayer).

Strategy (8 NeuronCores, SPMD, no collectives):
  - Sort edges by dest on host; split into 8 contiguous dest-node ranges with
    balanced edge counts. Each core owns every edge of its node range, so the
    segment_sum aggregate is computed wholly on-core (no all-reduce needed).
  - x[src] is fetched on-device with transpose-mode dma_gather from an fp16
    hi/lo split table (exact to ~2^-21), landing features-on-partitions.
  - x[dest] is never gathered: edges are grouped into 128-node dest windows;
    the x@W1b projection P_b of each window is computed on-device and the
    per-edge contribution is expanded with a one-hot indicator matmul
    (S_T[n,e] = dest[e]==n), generated on-device by DVE is_equal.
  - Edge MLP matmul1 uses the "dup-weight" trick: lhsT = [W_hi;W_hi] against
    the [hi;lo] stacked rhs computes (hi+lo)@W_hi in one K=128 matmul; a
    second [W_lo;W_lo] matmul restores full weight precision.
  - Matmul2 uses the data-stationary trick (lhsT = e_h chunk) so edge_out
    lands rows-on-partitions for the contiguous store and aggregation.
  - segment_sum = matmul with the S indicator ([e,n] layout), accumulated in
    PSUM per dest window.
  - Node MLP runs per-core on its 2560-node slab with the same tricks.

kernel(**inputs) takes FULL inputs and returns (x_out, edge_out) like the
reference.
"""

import numpy as np

import concourse.bass as bass
import concourse.bacc as bacc
import concourse.mybir as mybir
import concourse.tile as tile
from concourse.bass_utils import run_bass_kernel_spmd

dt = mybir.dt

N_NODES = 20000
N_EDGES = 640000
D = 64
H = 128
NCORES = 8
NC_PAD = 2560            # padded nodes per core
NWIN = NC_PAD // 128     # 20 dest windows of 128 nodes
CHUNK = 512              # edges per compute chunk (4 tiles of 128)
GCH = 2048               # edges per gather / streaming slab (4 chunks)
W_LO = True              # include W_lo matmuls (full weight precision)
AGG_LO = False           # include eo_lo in aggregation matmuls

_CACHE = {}
TRACE = False            # set True (with an NTFF hook installed) to profile
LAST_RESULT = None       # BassKernelResults of the most recent run


def _hl(a):
    hi = a.astype(np.float16)
    lo = (a.astype(np.float32) - hi.astype(np.float32)).astype(np.float16)
    return hi, lo


def _hl_cat(a):
    hi, lo = _hl(a)
    return np.concatenate([hi, lo], axis=1)


def _dup(w):
    """[64,128] f32 -> (hi_dup, lo_dup) [128,128] fp16 stacked twice on K."""
    hi, lo = _hl(w)
    return (
        np.concatenate([hi, hi], axis=0).copy(),
        np.concatenate([lo, lo], axis=0).copy(),
    )


def _wrap_idx(idx):
    """[E] -> [128, E/16] int16 in the dma_gather wrapped layout."""
    a = idx.astype(np.int16).reshape(-1, 16).T
    return np.tile(a, (8, 1)).copy()


def _make_plan(edge_index):
    src = np.asarray(edge_index[0]).astype(np.int32)
    dest = np.asarray(edge_index[1]).astype(np.int32)
    perm = np.argsort(dest, kind="stable").astype(np.int64)
    dsort = dest[perm]
    ssort = src[perm]
    counts = np.bincount(dest, minlength=N_NODES)
    cum = np.concatenate([[0], np.cumsum(counts)]).astype(np.int64)

    nb = [0]
    for k in range(1, NCORES):
        nb.append(int(np.searchsorted(cum, k * len(dsort) / NCORES)))
    nb.append(N_NODES)

    tiles_kw = np.zeros((NCORES, NWIN), np.int64)
    for k in range(NCORES):
        base = nb[k]
        assert nb[k + 1] - base <= NC_PAD, (k, nb)
        for w in range(NWIN):
            lo_n = base + 128 * w
            hi_n = min(base + 128 * (w + 1), nb[k + 1])
            ecnt = int(cum[hi_n] - cum[lo_n]) if hi_n > lo_n else 0
            tiles_kw[k, w] = (ecnt + 127) // 128
    T_w = np.maximum(tiles_kw.max(axis=0), 1)
    T_w = ((T_w + 3) // 4) * 4               # chunks never span windows
    NT = int(T_w.sum())
    NT_pad = ((NT + 15) // 16) * 16          # slabs of 16 tiles (GCH)
    T_w = T_w.copy()
    T_w[NWIN - 1] += NT_pad - NT             # trailing dummy tiles, S == 0
    NT = NT_pad
    E_CORE = NT * 128
    win_t0 = np.concatenate([[0], np.cumsum(T_w)]).astype(np.int64)

    slot_src = np.full((NCORES, E_CORE), N_NODES, np.int32)
    slot_orig = np.full((NCORES, E_CORE), -1, np.int64)
    slot_dwin = np.full((NCORES, E_CORE), 999.0, np.float32)
    for k in range(NCORES):
        base = nb[k]
        for w in range(NWIN):
            lo_n = base + 128 * w
            hi_n = min(base + 128 * (w + 1), nb[k + 1])
            if hi_n <= lo_n:
                continue
            e0, e1 = int(cum[lo_n]), int(cum[hi_n])
            cnt = e1 - e0
            s0 = int(win_t0[w]) * 128
            assert cnt <= int(T_w[w]) * 128
            slot_src[k, s0 : s0 + cnt] = ssort[e0:e1]
            slot_orig[k, s0 : s0 + cnt] = perm[e0:e1]
            slot_dwin[k, s0 : s0 + cnt] = (dsort[e0:e1] - lo_n).astype(np.float32)

    win_of_t = np.empty(NT, np.int64)
    for w in range(NWIN):
        win_of_t[win_t0[w] : win_t0[w + 1]] = w

    return dict(
        nb=nb, T_w=T_w, NT=NT, E_CORE=E_CORE, win_t0=win_t0, win_of_t=win_of_t,
        slot_src=slot_src, slot_orig=slot_orig, slot_dwin=slot_dwin,
    )


def _build_bass(NT, win_t0, win_of_t, has_be2):
    E_CORE = NT * 128
    NSLAB = E_CORE // GCH
    nc = bacc.Bacc("TRN2", target_bir_lowering=False, debug=False,
                   num_devices=NCORES)

    # ---- DRAM I/O ----
    xhl_d = nc.dram_tensor("xhl", [N_NODES + 1, 128], dt.float16, kind="ExternalInput")
    sidx_d = nc.dram_tensor("sidx", [128, E_CORE // 16], dt.int16, kind="ExternalInput")
    eahl_d = nc.dram_tensor("eahl", [E_CORE, 128], dt.float16, kind="ExternalInput")
    dwin_d = nc.dram_tensor("dwin", [128, NT], dt.float16, kind="ExternalInput")
    dwf_d = nc.dram_tensor("dwf", [NSLAB, GCH], dt.float16, kind="ExternalInput")
    iota_d = nc.dram_tensor("iotar", [128, 128], dt.float16, kind="ExternalInput")
    iotac_d = nc.dram_tensor("iotac", [128, 1], dt.float32, kind="ExternalInput")
    ones_d = nc.dram_tensor("ones1", [1, 128], dt.float16, kind="ExternalInput")
    ident_d = nc.dram_tensor("ident", [128, 128], dt.float32, kind="ExternalInput")
    xn_d = nc.dram_tensor("xn", [NC_PAD, 128], dt.float16, kind="ExternalInput")
    fn_d = nc.dram_tensor("fn", [NC_PAD, 128], dt.float16, kind="ExternalInput")
    w_names = [
        ("w1a_hi", [128, 128]), ("w1a_lo", [128, 128]),
        ("w1bd_hi", [128, 128]), ("w1bd_lo", [128, 128]),
        ("w1c_hi", [128, 128]), ("w1c_lo", [128, 128]),
        ("we2_hi", [128, 64]), ("we2_lo", [128, 64]),
        ("wn1a_hi", [128, 128]), ("wn1a_lo", [128, 128]),
        ("wn1c_hi", [128, 128]), ("wn1c_lo", [128, 128]),
        ("wn2_hi", [128, 64]), ("wn2_lo", [128, 64]),
    ]
    w_d = {n: nc.dram_tensor(n, s, dt.float16, kind="ExternalInput") for n, s in w_names}
    wn1b_d = nc.dram_tensor("wn1b", [64, 128], dt.float32, kind="ExternalInput")
    b1_d = nc.dram_tensor("b1c", [128, 1], dt.float32, kind="ExternalInput")
    b2_d = nc.dram_tensor("b2r", [128, 256], dt.float32, kind="ExternalInput")
    bn1_d = nc.dram_tensor("bn1c", [128, 1], dt.float32, kind="ExternalInput")
    bn2_d = nc.dram_tensor("bn2r", [128, 256], dt.float32, kind="ExternalInput")

    o_eo = nc.dram_tensor("o_eo", [E_CORE, 64], dt.float32, kind="ExternalOutput")
    o_x = nc.dram_tensor("o_x", [NC_PAD, 64], dt.float32, kind="ExternalOutput")

    RELU = mybir.ActivationFunctionType.Relu
    COPY = mybir.ActivationFunctionType.Copy

    with tile.TileContext(nc) as tc:
        with (
            tc.tile_pool(name="consts", bufs=1) as cp,
            tc.tile_pool(name="gather", bufs=2) as gp,
            tc.tile_pool(name="work", bufs=3) as wp,
            tc.tile_pool(name="psh", bufs=2, space="PSUM") as psh,
            tc.tile_pool(name="pse", bufs=2, space="PSUM") as pse,
            tc.tile_pool(name="psagg", bufs=2, space="PSUM") as psagg,
            tc.tile_pool(name="psdw", bufs=2, space="PSUM") as psdw,
        ):
            # ---- constants into SBUF ----
            def load_const(dram, shape, dtype):
                t = cp.tile(shape, dtype, tag=dram.name + "_c")
                nc.sync.dma_start(t[:], dram[:])
                return t

            wt = {n: load_const(w_d[n], s, dt.float16) for n, s in w_names}
            wn1b_t = load_const(wn1b_d, [64, 128], dt.float32)
            b1_t = load_const(b1_d, [128, 1], dt.float32)
            b2_t = load_const(b2_d, [128, 256], dt.float32)
            bn1_t = load_const(bn1_d, [128, 1], dt.float32)
            bn2_t = load_const(bn2_d, [128, 256], dt.float32)
            iota_t = load_const(iota_d, [128, 128], dt.float16)
            iotac_t = load_const(iotac_d, [128, 1], dt.float32)
            ones_t = load_const(ones_d, [1, 128], dt.float16)
            ident_t = load_const(ident_d, [128, 128], dt.float32)
            sidx_t = load_const(sidx_d, [128, E_CORE // 16], dt.int16)
            dwin_t = load_const(dwin_d, [128, NT], dt.float16)

            agg_sb = cp.tile([128, NWIN * 64], dt.float32, tag="agg_sb")
            xt = cp.tile([128, NC_PAD], dt.float16, tag="xt")
            nc.sync.dma_start_transpose(xt[:], xn_d[:])
            ft = cp.tile([128, NC_PAD], dt.float16, tag="ft")
            nc.sync.dma_start_transpose(ft[:], fn_d[:])

            # ---- per-window x@W1b projections (P_b), fp16 hi/lo ----
            pb_hi = cp.tile([128, NWIN * 128], dt.float16, tag="pb_hi")
            pb_lo = cp.tile([128, NWIN * 128], dt.float16, tag="pb_lo")
            for w in range(NWIN):
                ps = pse.tile([128, 128], dt.float32, tag="pe")
                xw = xt[:, 128 * w : 128 * (w + 1)]
                nc.tensor.matmul(ps[:], xw, wt["w1bd_hi"][:], start=True, stop=False)
                nc.tensor.matmul(ps[:], xw, wt["w1bd_lo"][:], start=False, stop=True)
                sl = slice(128 * w, 128 * (w + 1))
                nc.scalar.activation(pb_hi[:, sl], ps[:], COPY)
                nc.vector.tensor_tensor(pb_lo[:, sl], ps[:], pb_hi[:, sl],
                                        mybir.AluOpType.subtract)

            cur_agg = None

            # ---- edge phase ----
            for g in range(NSLAB):
                xs_big = gp.tile([128, GCH], dt.float16, tag="xs")
                nc.gpsimd.dma_gather(
                    xs_big[:].rearrange("p (a n) -> p a n", a=1),
                    xhl_d[:],
                    sidx_t[:, g * (GCH // 16) : (g + 1) * (GCH // 16)],
                    GCH, GCH, 128, transpose=True, single_packet=False,
                )
                ea_big = gp.tile([128, GCH], dt.float16, tag="ea")
                nc.sync.dma_start_transpose(
                    ea_big[:], eahl_d[g * GCH : (g + 1) * GCH, :]
                )
                dwr = gp.tile([1, GCH], dt.float16, tag="dwr")
                nc.sync.dma_start(dwr[:], dwf_d[g : g + 1, :])

                for cc in range(GCH // CHUNK):
                    ch = g * (GCH // CHUNK) + cc
                    sl = slice(cc * CHUNK, (cc + 1) * CHUNK)
                    XS, EA = xs_big[:, sl], ea_big[:, sl]

                    # dest one-hot, both layouts
                    s_t = wp.tile([128, CHUNK], dt.float16, tag="s")
                    nc.vector.tensor_tensor(
                        s_t[:].rearrange("p (a n) -> p a n", a=4),
                        iota_t[:].unsqueeze(1).broadcast_to([128, 4, 128]),
                        dwin_t[:, ch * 4 : ch * 4 + 4].unsqueeze(2)
                        .broadcast_to([128, 4, 128]),
                        mybir.AluOpType.is_equal,
                    )
                    pdw = psdw.tile([128, CHUNK], dt.float32, tag="dw")
                    nc.tensor.matmul(pdw[:], ones_t[:], dwr[:, sl],
                                     start=True, stop=True)
                    st_t = wp.tile([128, CHUNK], dt.float16, tag="st")
                    nc.vector.tensor_scalar(
                        st_t[:], pdw[:], iotac_t[:, 0:1], None,
                        mybir.AluOpType.is_equal,
                    )

                    # edge MLP layer 1 — x[dest] contribution first via
                    # windowed projection expansion (region starts), then the
                    # full-range src/edge_attr matmuls accumulate on top.
                    ph = psh.tile([128, CHUNK], dt.float32, tag="ph")
                    wch = int(win_of_t[ch * 4])
                    assert all(int(win_of_t[ch * 4 + i]) == wch for i in range(4))
                    psl = slice(128 * wch, 128 * (wch + 1))
                    nc.tensor.matmul(ph[:], pb_hi[:, psl], st_t[:],
                                     start=True, stop=False)
                    nc.tensor.matmul(ph[:], pb_lo[:, psl], st_t[:],
                                     start=False, stop=False)
                    mm1 = [("w1a_hi", XS), ("w1c_hi", EA)]
                    if W_LO:
                        mm1 += [("w1a_lo", XS), ("w1c_lo", EA)]
                    for i, (wn, rhs) in enumerate(mm1):
                        nc.tensor.matmul(ph[:], wt[wn][:], rhs,
                                         start=False, stop=(i == len(mm1) - 1))

                    e_hi = wp.tile([128, CHUNK], dt.float16, tag="eh")
                    nc.scalar.activation(e_hi[:], ph[:], RELU, bias=b1_t[:, 0:1])

                    # edge MLP layer 2 (data-stationary): eo[128e,64] per tile
                    pe = pse.tile([128, 256], dt.float32, tag="pe")
                    for i in range(4):
                        lhsT = e_hi[:, 128 * i : 128 * (i + 1)]
                        nc.tensor.matmul(pe[:, 64 * i : 64 * i + 64], lhsT,
                                         wt["we2_hi"][:], start=True, stop=False)
                        nc.tensor.matmul(pe[:, 64 * i : 64 * i + 64], lhsT,
                                         wt["we2_lo"][:], start=False, stop=True)

                    # edge_out store (+ be2 if nonzero)
                    eo32 = wp.tile([128, 256], dt.float32, tag="eo32")
                    if has_be2:
                        nc.vector.tensor_tensor(eo32[:], pe[:], b2_t[:],
                                                mybir.AluOpType.add)
                    else:
                        nc.scalar.activation(eo32[:], pe[:], COPY)
                    nc.sync.dma_start(
                        o_eo[ch * CHUNK : (ch + 1) * CHUNK, :]
                        .rearrange("(t p) f -> p t f", p=128),
                        eo32[:].rearrange("p (t f) -> p t f", t=4),
                    )

                    # fp16 (hi[/lo]) of pre-bias edge_out for aggregation
                    eo_hi = wp.tile([128, 256], dt.float16, tag="eohi")
                    nc.scalar.activation(eo_hi[:], pe[:], COPY)
                    if AGG_LO:
                        eo_lo = wp.tile([128, 256], dt.float16, tag="eolo")
                        nc.vector.tensor_tensor(eo_lo[:], pe[:], eo_hi[:],
                                                mybir.AluOpType.subtract)

                    # aggregation matmuls
                    for i in range(4):
                        t = ch * 4 + i
                        w = int(win_of_t[t])
                        first = t == int(win_t0[w])
                        last = t == int(win_t0[w + 1]) - 1
                        if first:
                            cur_agg = psagg.tile([128, 64], dt.float32, tag="pagg")
                        S_i = s_t[:, 128 * i : 128 * (i + 1)]
                        nc.tensor.matmul(cur_agg[:], S_i,
                                         eo_hi[:, 64 * i : 64 * i + 64],
                                         start=first, stop=(last and not AGG_LO))
                        if AGG_LO:
                            nc.tensor.matmul(cur_agg[:], S_i,
                                             eo_lo[:, 64 * i : 64 * i + 64],
                                             start=False, stop=last)
                        if last:
                            nc.vector.tensor_copy(
                                agg_sb[:, w * 64 : (w + 1) * 64], cur_agg[:]
                            )

            # ---- node phase ----
            aggT = cp.tile([64, NC_PAD], dt.float32, tag="aggT")
            for w in range(NWIN):
                pt = psdw.tile([64, 128], dt.float32, tag="dw")
                nc.tensor.transpose(pt[:], agg_sb[:, w * 64 : (w + 1) * 64],
                                    ident_t[:])
                nc.vector.tensor_copy(aggT[:, w * 128 : (w + 1) * 128], pt[:])

            for c in range(NC_PAD // CHUNK):
                sl = slice(c * CHUNK, (c + 1) * CHUNK)
                pn = psh.tile([128, CHUNK], dt.float32, tag="ph")
                mms = [("wn1a_hi", xt[:, sl]), ("wn1c_hi", ft[:, sl])]
                if W_LO:
                    mms += [("wn1a_lo", xt[:, sl]), ("wn1c_lo", ft[:, sl])]
                for i, (wn, rhs) in enumerate(mms):
                    nc.tensor.matmul(pn[:], wt[wn][:], rhs,
                                     start=(i == 0), stop=False)
                nc.tensor.matmul(pn[:], wn1b_t[:], aggT[:, sl],
                                 start=False, stop=True)
                n_hi = wp.tile([128, CHUNK], dt.float16, tag="eh")
                nc.scalar.activation(n_hi[:], pn[:], RELU, bias=bn1_t[:, 0:1])
                px = pse.tile([128, 256], dt.float32, tag="pe")
                for i in range(4):
                    lhsT = n_hi[:, 128 * i : 128 * (i + 1)]
                    nc.tensor.matmul(px[:, 64 * i : 64 * i + 64], lhsT,
                                     wt["wn2_hi"][:], start=True, stop=False)
                    nc.tensor.matmul(px[:, 64 * i : 64 * i + 64], lhsT,
                                     wt["wn2_lo"][:], start=False, stop=True)
                x32 = wp.tile([128, 256], dt.float32, tag="eo32")
                nc.vector.tensor_tensor(x32[:], px[:], bn2_t[:],
                                        mybir.AluOpType.add)
                nc.sync.dma_start(
                    o_x[c * CHUNK : (c + 1) * CHUNK, :]
                    .rearrange("(t p) f -> p t f", p=128),
                    x32[:].rearrange("p (t f) -> p t f", t=4),
                )
    nc.finalize()
    return nc


def prepare(x, edge_index, edge_attr, f, We1, be1, We2, be2, Wn1, bn1, Wn2, bn2):
    x = np.asarray(x, np.float32)
    edge_attr = np.asarray(edge_attr, np.float32)
    f = np.asarray(f, np.float32)
    We1 = np.asarray(We1, np.float32); be1 = np.asarray(be1, np.float32)
    We2 = np.asarray(We2, np.float32); be2 = np.asarray(be2, np.float32)
    Wn1 = np.asarray(Wn1, np.float32); bn1 = np.asarray(bn1, np.float32)
    Wn2 = np.asarray(Wn2, np.float32); bn2 = np.asarray(bn2, np.float32)

    plan = _make_plan(edge_index)
    NT = plan["NT"]
    E_CORE = plan["E_CORE"]
    nb = plan["nb"]
    NSLAB = E_CORE // GCH
    has_be2 = bool(np.any(be2))

    key = (NT, tuple(int(t) for t in plan["T_w"]), has_be2, W_LO, AGG_LO)
    if key not in _CACHE:
        _CACHE[key] = _build_bass(NT, plan["win_t0"], plan["win_of_t"], has_be2)
    nc = _CACHE[key]

    # ---- replicated arrays ----
    xhl = np.zeros((N_NODES + 1, 128), np.float16)
    xhl[:N_NODES] = _hl_cat(x)
    ea_hl_full = _hl_cat(edge_attr)
    f_hl_full = _hl_cat(f)
    x_hl_full = xhl[:N_NODES]

    w1a_hi, w1a_lo = _dup(We1[0:64])
    w1b_hi16, w1b_lo16 = _hl(We1[64:128])      # [64,128] fp16
    w1bd_hi = np.concatenate([w1b_hi16, w1b_hi16], axis=0)
    w1bd_lo = np.concatenate([w1b_lo16, np.zeros_like(w1b_lo16)], axis=0)
    w1c_hi, w1c_lo = _dup(We1[128:192])
    we2_hi, we2_lo = _hl(We2)
    wn1a_hi, wn1a_lo = _dup(Wn1[0:64])
    wn1c_hi, wn1c_lo = _dup(Wn1[128:192])
    wn1b = Wn1[64:128].copy()
    wn2_hi, wn2_lo = _hl(Wn2)

    iota_rep = np.tile(np.arange(128, dtype=np.float16), (128, 1))
    iotac = np.arange(128, dtype=np.float32).reshape(128, 1)
    ones1 = np.ones((1, 128), np.float16)
    ident = np.eye(128, dtype=np.float32)
    b1c = be1.reshape(128, 1)
    b2r = np.tile(be2.reshape(1, 64), (128, 4))
    bn1c = bn1.reshape(128, 1)
    bn2r = np.tile(bn2.reshape(1, 64), (128, 4))

    rep = dict(
        xhl=xhl, iotar=iota_rep, iotac=iotac, ones1=ones1, ident=ident,
        w1a_hi=w1a_hi, w1a_lo=w1a_lo, w1bd_hi=w1bd_hi, w1bd_lo=w1bd_lo,
        w1c_hi=w1c_hi, w1c_lo=w1c_lo, we2_hi=we2_hi, we2_lo=we2_lo,
        wn1a_hi=wn1a_hi, wn1a_lo=wn1a_lo, wn1c_hi=wn1c_hi, wn1c_lo=wn1c_lo,
        wn2_hi=wn2_hi, wn2_lo=wn2_lo, wn1b=wn1b,
        b1c=b1c, b2r=b2r, bn1c=bn1c, bn2r=bn2r,
    )

    in_maps = []
    for k in range(NCORES):
        sor = plan["slot_orig"][k]
        m = sor >= 0
        eahl = np.zeros((E_CORE, 128), np.float16)
        eahl[m] = ea_hl_full[sor[m]]
        base = nb[k]
        hi_n = min(base + NC_PAD, N_NODES)
        xn = np.zeros((NC_PAD, 128), np.float16)
        xn[: hi_n - base] = x_hl_full[base:hi_n]
        fn = np.zeros((NC_PAD, 128), np.float16)
        fn[: hi_n - base] = f_hl_full[base:hi_n]
        dwin16 = plan["slot_dwin"][k].astype(np.float16)
        im = dict(rep)
        im.update(
            sidx=_wrap_idx(plan["slot_src"][k]),
            eahl=eahl,
            dwin=dwin16.reshape(NT, 128).T.copy(),
            dwf=dwin16.reshape(NSLAB, GCH).copy(),
            xn=xn, fn=fn,
        )
        in_maps.append(im)

    return nc, plan, in_maps


def kernel(**inputs):
    nc, plan, in_maps = prepare(**inputs)
    nb = plan["nb"]
    res = run_bass_kernel_spmd(nc, in_maps, core_ids=list(range(NCORES)),
                               trace=TRACE)
    global LAST_RESULT
    LAST_RESULT = res

    edge_out = np.empty((N_EDGES, D), np.float32)
    x_out = np.empty((N_NODES, D), np.float32)
    for k in range(NCORES):
        sor = plan["slot_orig"][k]
        m = sor >= 0
        edge_out[sor[m]] = res.results[k]["o_eo"][m]
        x_out[nb[k] : nb[k + 1]] = res.results[k]["o_x"][: nb[k + 1] - nb[k]]
    return (x_out, edge_out)


# revision 10
# speedup vs baseline: 1.7278x; 1.7278x over previous
"""Trainium2 Bass kernel for nn_MetaLayer (GNN message passing MetaLayer).

Strategy (8 NeuronCores, SPMD, no collectives):
  - Sort edges by dest on host; split into 8 contiguous dest-node ranges with
    balanced edge counts. Each core owns every edge of its node range, so the
    segment_sum aggregate is computed wholly on-core (no all-reduce needed).
  - x[src] is fetched on-device with transpose-mode dma_gather from an fp16
    hi/lo split table (exact to ~2^-21), landing features-on-partitions.
  - x[dest] is never gathered: edges are grouped into 128-node dest windows;
    the x@W1b projection P_b of each window is computed on-device and the
    per-edge contribution is expanded with a one-hot indicator matmul
    (S_T[n,e] = dest[e]==n), generated on-device by DVE is_equal.
  - Edge MLP matmul1 uses the "dup-weight" trick: lhsT = [W_hi;W_hi] against
    the [hi;lo] stacked rhs computes (hi+lo)@W_hi in one K=128 matmul; a
    second [W_lo;W_lo] matmul restores full weight precision.
  - Matmul2 uses the data-stationary trick (lhsT = e_h chunk) so edge_out
    lands rows-on-partitions for the contiguous store and aggregation.
  - segment_sum = matmul with the S indicator ([e,n] layout), accumulated in
    PSUM per dest window.
  - Node MLP runs per-core on its 2560-node slab with the same tricks.

kernel(**inputs) takes FULL inputs and returns (x_out, edge_out) like the
reference.
"""

import numpy as np

import concourse.bass as bass
import concourse.bacc as bacc
import concourse.mybir as mybir
import concourse.tile as tile
from concourse.bass_utils import run_bass_kernel_spmd

dt = mybir.dt

N_NODES = 20000
N_EDGES = 640000
D = 64
H = 128
NCORES = 8
NC_PAD = 2560            # padded nodes per core
NWIN = NC_PAD // 128     # 20 dest windows of 128 nodes
CHUNK = 512              # edges per compute chunk (4 tiles of 128)
GCH = 2048               # edges per gather / streaming slab (4 chunks)
W_LO = True              # include W_lo matmuls (full weight precision)
AGG_LO = False           # include eo_lo in aggregation matmuls

_CACHE = {}
TRACE = False            # set True (with an NTFF hook installed) to profile
LAST_RESULT = None       # BassKernelResults of the most recent run


def _hl(a):
    hi = a.astype(np.float16)
    lo = (a.astype(np.float32) - hi.astype(np.float32)).astype(np.float16)
    return hi, lo


def _hl_cat(a):
    hi, lo = _hl(a)
    return np.concatenate([hi, lo], axis=1)


def _dup(w):
    """[64,128] f32 -> (hi_dup, lo_dup) [128,128] fp16 stacked twice on K."""
    hi, lo = _hl(w)
    return (
        np.concatenate([hi, hi], axis=0).copy(),
        np.concatenate([lo, lo], axis=0).copy(),
    )


def _wrap_idx(idx):
    """[E] -> [128, E/16] int16 in the dma_gather wrapped layout."""
    a = idx.astype(np.int16).reshape(-1, 16).T
    return np.tile(a, (8, 1)).copy()


def _make_plan(edge_index):
    src = np.asarray(edge_index[0]).astype(np.int32)
    dest = np.asarray(edge_index[1]).astype(np.int32)
    perm = np.argsort(dest, kind="stable").astype(np.int64)
    dsort = dest[perm]
    ssort = src[perm]
    counts = np.bincount(dest, minlength=N_NODES)
    cum = np.concatenate([[0], np.cumsum(counts)]).astype(np.int64)

    nb = [0]
    for k in range(1, NCORES):
        nb.append(int(np.searchsorted(cum, k * len(dsort) / NCORES)))
    nb.append(N_NODES)

    tiles_kw = np.zeros((NCORES, NWIN), np.int64)
    for k in range(NCORES):
        base = nb[k]
        assert nb[k + 1] - base <= NC_PAD, (k, nb)
        for w in range(NWIN):
            lo_n = base + 128 * w
            hi_n = min(base + 128 * (w + 1), nb[k + 1])
            ecnt = int(cum[hi_n] - cum[lo_n]) if hi_n > lo_n else 0
            tiles_kw[k, w] = (ecnt + 127) // 128
    T_w = np.maximum(tiles_kw.max(axis=0), 1)
    T_w = ((T_w + 3) // 4) * 4               # chunks never span windows
    NT = int(T_w.sum())
    NT_pad = ((NT + 15) // 16) * 16          # slabs of 16 tiles (GCH)
    T_w = T_w.copy()
    T_w[NWIN - 1] += NT_pad - NT             # trailing dummy tiles, S == 0
    NT = NT_pad
    E_CORE = NT * 128
    win_t0 = np.concatenate([[0], np.cumsum(T_w)]).astype(np.int64)

    slot_src = np.full((NCORES, E_CORE), N_NODES, np.int32)
    slot_orig = np.full((NCORES, E_CORE), -1, np.int64)
    slot_dwin = np.full((NCORES, E_CORE), 999.0, np.float32)
    for k in range(NCORES):
        base = nb[k]
        for w in range(NWIN):
            lo_n = base + 128 * w
            hi_n = min(base + 128 * (w + 1), nb[k + 1])
            if hi_n <= lo_n:
                continue
            e0, e1 = int(cum[lo_n]), int(cum[hi_n])
            cnt = e1 - e0
            s0 = int(win_t0[w]) * 128
            assert cnt <= int(T_w[w]) * 128
            slot_src[k, s0 : s0 + cnt] = ssort[e0:e1]
            slot_orig[k, s0 : s0 + cnt] = perm[e0:e1]
            slot_dwin[k, s0 : s0 + cnt] = (dsort[e0:e1] - lo_n).astype(np.float32)

    win_of_t = np.empty(NT, np.int64)
    for w in range(NWIN):
        win_of_t[win_t0[w] : win_t0[w + 1]] = w

    return dict(
        nb=nb, T_w=T_w, NT=NT, E_CORE=E_CORE, win_t0=win_t0, win_of_t=win_of_t,
        slot_src=slot_src, slot_orig=slot_orig, slot_dwin=slot_dwin,
    )


def _build_bass(NT, win_t0, win_of_t, has_be2):
    E_CORE = NT * 128
    NSLAB = E_CORE // GCH
    nc = bacc.Bacc("TRN2", target_bir_lowering=False, debug=False,
                   num_devices=NCORES)

    # ---- DRAM I/O ----
    xhl_d = nc.dram_tensor("xhl", [N_NODES + 1, 128], dt.float16, kind="ExternalInput")
    sidx_d = nc.dram_tensor("sidx", [128, E_CORE // 16], dt.int16, kind="ExternalInput")
    eahl_d = nc.dram_tensor("eahl", [E_CORE, 128], dt.float16, kind="ExternalInput")
    dwin_d = nc.dram_tensor("dwin", [128, NT], dt.float16, kind="ExternalInput")
    dwf_d = nc.dram_tensor("dwf", [NSLAB, GCH], dt.float16, kind="ExternalInput")
    iota_d = nc.dram_tensor("iotar", [128, 128], dt.float16, kind="ExternalInput")
    iotac_d = nc.dram_tensor("iotac", [128, 1], dt.float32, kind="ExternalInput")
    ones_d = nc.dram_tensor("ones1", [1, 128], dt.float16, kind="ExternalInput")
    ident_d = nc.dram_tensor("ident", [128, 128], dt.float32, kind="ExternalInput")
    xn_d = nc.dram_tensor("xn", [NC_PAD, 128], dt.float16, kind="ExternalInput")
    fn_d = nc.dram_tensor("fn", [NC_PAD, 128], dt.float16, kind="ExternalInput")
    w_names = [
        ("w1a_hi", [128, 128]), ("w1a_lo", [128, 128]),
        ("w1bd_hi", [128, 128]), ("w1bd_lo", [128, 128]),
        ("w1c_hi", [128, 128]), ("w1c_lo", [128, 128]),
        ("we2_hi", [128, 64]), ("we2_lo", [128, 64]),
        ("wn1a_hi", [128, 128]), ("wn1a_lo", [128, 128]),
        ("wn1c_hi", [128, 128]), ("wn1c_lo", [128, 128]),
        ("wn2_hi", [128, 64]), ("wn2_lo", [128, 64]),
    ]
    w_d = {n: nc.dram_tensor(n, s, dt.float16, kind="ExternalInput") for n, s in w_names}
    wn1b_d = nc.dram_tensor("wn1b", [64, 128], dt.float32, kind="ExternalInput")
    b1_d = nc.dram_tensor("b1c", [128, 1], dt.float32, kind="ExternalInput")
    b2_d = nc.dram_tensor("b2r", [128, 256], dt.float32, kind="ExternalInput")
    bn1_d = nc.dram_tensor("bn1c", [128, 1], dt.float32, kind="ExternalInput")
    bn2_d = nc.dram_tensor("bn2r", [128, 256], dt.float32, kind="ExternalInput")

    o_eo = nc.dram_tensor("o_eo", [E_CORE, 64], dt.float32, kind="ExternalOutput")
    o_x = nc.dram_tensor("o_x", [NC_PAD, 64], dt.float32, kind="ExternalOutput")

    RELU = mybir.ActivationFunctionType.Relu
    COPY = mybir.ActivationFunctionType.Copy

    with tile.TileContext(nc) as tc:
        with (
            tc.tile_pool(name="consts", bufs=1) as cp,
            tc.tile_pool(name="gather", bufs=4) as gp,
            tc.tile_pool(name="work", bufs=6) as wp,
            tc.tile_pool(name="psh", bufs=2, space="PSUM") as psh,
            tc.tile_pool(name="pse", bufs=2, space="PSUM") as pse,
            tc.tile_pool(name="psagg", bufs=2, space="PSUM") as psagg,
            tc.tile_pool(name="psdw", bufs=2, space="PSUM") as psdw,
        ):
            # ---- constants into SBUF ----
            def load_const(dram, shape, dtype):
                t = cp.tile(shape, dtype, tag=dram.name + "_c")
                nc.sync.dma_start(t[:], dram[:])
                return t

            wt = {n: load_const(w_d[n], s, dt.float16) for n, s in w_names}
            wn1b_t = load_const(wn1b_d, [64, 128], dt.float32)
            b1_t = load_const(b1_d, [128, 1], dt.float32)
            b2_t = load_const(b2_d, [128, 256], dt.float32)
            bn1_t = load_const(bn1_d, [128, 1], dt.float32)
            bn2_t = load_const(bn2_d, [128, 256], dt.float32)
            iota_t = load_const(iota_d, [128, 128], dt.float16)
            iotac_t = load_const(iotac_d, [128, 1], dt.float32)
            ones_t = load_const(ones_d, [1, 128], dt.float16)
            ident_t = load_const(ident_d, [128, 128], dt.float32)
            sidx_t = load_const(sidx_d, [128, E_CORE // 16], dt.int16)
            dwin_t = load_const(dwin_d, [128, NT], dt.float16)

            agg_sb = cp.tile([128, NWIN * 64], dt.float32, tag="agg_sb")
            xt = cp.tile([128, NC_PAD], dt.float16, tag="xt")
            nc.sync.dma_start_transpose(xt[:], xn_d[:])
            ft = cp.tile([128, NC_PAD], dt.float16, tag="ft")
            nc.sync.dma_start_transpose(ft[:], fn_d[:])

            # ---- per-window x@W1b projections (P_b), fp16 hi/lo ----
            pb_hi = cp.tile([128, NWIN * 128], dt.float16, tag="pb_hi")
            pb_lo = cp.tile([128, NWIN * 128], dt.float16, tag="pb_lo")
            for w in range(NWIN):
                ps = pse.tile([128, 128], dt.float32, tag="pe")
                xw = xt[:, 128 * w : 128 * (w + 1)]
                nc.tensor.matmul(ps[:], xw, wt["w1bd_hi"][:], start=True, stop=False)
                nc.tensor.matmul(ps[:], xw, wt["w1bd_lo"][:], start=False, stop=True)
                sl = slice(128 * w, 128 * (w + 1))
                nc.scalar.activation(pb_hi[:, sl], ps[:], COPY)
                nc.vector.tensor_tensor(pb_lo[:, sl], ps[:], pb_hi[:, sl],
                                        mybir.AluOpType.subtract)

            cur_agg = None

            # ---- edge phase ----
            for g in range(NSLAB):
                xs_big = gp.tile([128, GCH], dt.float16, tag="xs")
                nc.gpsimd.dma_gather(
                    xs_big[:].rearrange("p (a n) -> p a n", a=1),
                    xhl_d[:],
                    sidx_t[:, g * (GCH // 16) : (g + 1) * (GCH // 16)],
                    GCH, GCH, 128, transpose=True, single_packet=False,
                )
                ea_big = gp.tile([128, GCH], dt.float16, tag="ea")
                nc.sync.dma_start_transpose(
                    ea_big[:], eahl_d[g * GCH : (g + 1) * GCH, :]
                )
                dwr = gp.tile([1, GCH], dt.float16, tag="dwr")
                nc.sync.dma_start(dwr[:], dwf_d[g : g + 1, :])

                for cc in range(GCH // CHUNK):
                    ch = g * (GCH // CHUNK) + cc
                    sl = slice(cc * CHUNK, (cc + 1) * CHUNK)
                    XS, EA = xs_big[:, sl], ea_big[:, sl]

                    # dest one-hot, both layouts
                    s_t = wp.tile([128, CHUNK], dt.float16, tag="s")
                    nc.vector.tensor_tensor(
                        s_t[:].rearrange("p (a n) -> p a n", a=4),
                        iota_t[:].unsqueeze(1).broadcast_to([128, 4, 128]),
                        dwin_t[:, ch * 4 : ch * 4 + 4].unsqueeze(2)
                        .broadcast_to([128, 4, 128]),
                        mybir.AluOpType.is_equal,
                    )
                    pdw = psdw.tile([128, CHUNK], dt.float32, tag="dw")
                    nc.tensor.matmul(pdw[:], ones_t[:], dwr[:, sl],
                                     start=True, stop=True)
                    st_t = wp.tile([128, CHUNK], dt.float16, tag="st")
                    nc.vector.tensor_scalar(
                        st_t[:], pdw[:], iotac_t[:, 0:1], None,
                        mybir.AluOpType.is_equal,
                    )

                    # edge MLP layer 1 — x[dest] contribution first via
                    # windowed projection expansion (region starts), then the
                    # full-range src/edge_attr matmuls accumulate on top.
                    ph = psh.tile([128, CHUNK], dt.float32, tag="ph")
                    wch = int(win_of_t[ch * 4])
                    assert all(int(win_of_t[ch * 4 + i]) == wch for i in range(4))
                    psl = slice(128 * wch, 128 * (wch + 1))
                    nc.tensor.matmul(ph[:], pb_hi[:, psl], st_t[:],
                                     start=True, stop=False)
                    nc.tensor.matmul(ph[:], pb_lo[:, psl], st_t[:],
                                     start=False, stop=False)
                    mm1 = [("w1a_hi", XS), ("w1c_hi", EA)]
                    if W_LO:
                        mm1 += [("w1a_lo", XS), ("w1c_lo", EA)]
                    for i, (wn, rhs) in enumerate(mm1):
                        nc.tensor.matmul(ph[:], wt[wn][:], rhs,
                                         start=False, stop=(i == len(mm1) - 1))

                    e_hi = wp.tile([128, CHUNK], dt.float16, tag="eh")
                    nc.scalar.activation(e_hi[:], ph[:], RELU, bias=b1_t[:, 0:1])

                    # edge MLP layer 2 (data-stationary): eo[128e,64] per tile
                    pe = pse.tile([128, 256], dt.float32, tag="pe")
                    for i in range(4):
                        lhsT = e_hi[:, 128 * i : 128 * (i + 1)]
                        nc.tensor.matmul(pe[:, 64 * i : 64 * i + 64], lhsT,
                                         wt["we2_hi"][:], start=True, stop=False)
                        nc.tensor.matmul(pe[:, 64 * i : 64 * i + 64], lhsT,
                                         wt["we2_lo"][:], start=False, stop=True)

                    # edge_out store (+ be2 if nonzero)
                    eo32 = wp.tile([128, 256], dt.float32, tag="eo32")
                    if has_be2:
                        nc.vector.tensor_tensor(eo32[:], pe[:], b2_t[:],
                                                mybir.AluOpType.add)
                    else:
                        nc.scalar.activation(eo32[:], pe[:], COPY)
                    nc.sync.dma_start(
                        o_eo[ch * CHUNK : (ch + 1) * CHUNK, :]
                        .rearrange("(t p) f -> p t f", p=128),
                        eo32[:].rearrange("p (t f) -> p t f", t=4),
                    )

                    # fp16 (hi[/lo]) of pre-bias edge_out for aggregation
                    eo_hi = wp.tile([128, 256], dt.float16, tag="eohi")
                    nc.scalar.activation(eo_hi[:], pe[:], COPY)
                    if AGG_LO:
                        eo_lo = wp.tile([128, 256], dt.float16, tag="eolo")
                        nc.vector.tensor_tensor(eo_lo[:], pe[:], eo_hi[:],
                                                mybir.AluOpType.subtract)

                    # aggregation matmuls
                    for i in range(4):
                        t = ch * 4 + i
                        w = int(win_of_t[t])
                        first = t == int(win_t0[w])
                        last = t == int(win_t0[w + 1]) - 1
                        if first:
                            cur_agg = psagg.tile([128, 64], dt.float32, tag="pagg")
                        S_i = s_t[:, 128 * i : 128 * (i + 1)]
                        nc.tensor.matmul(cur_agg[:], S_i,
                                         eo_hi[:, 64 * i : 64 * i + 64],
                                         start=first, stop=(last and not AGG_LO))
                        if AGG_LO:
                            nc.tensor.matmul(cur_agg[:], S_i,
                                             eo_lo[:, 64 * i : 64 * i + 64],
                                             start=False, stop=last)
                        if last:
                            nc.vector.tensor_copy(
                                agg_sb[:, w * 64 : (w + 1) * 64], cur_agg[:]
                            )

            # ---- node phase ----
            aggT = cp.tile([64, NC_PAD], dt.float32, tag="aggT")
            for w in range(NWIN):
                pt = psdw.tile([64, 128], dt.float32, tag="dw")
                nc.tensor.transpose(pt[:], agg_sb[:, w * 64 : (w + 1) * 64],
                                    ident_t[:])
                nc.vector.tensor_copy(aggT[:, w * 128 : (w + 1) * 128], pt[:])

            for c in range(NC_PAD // CHUNK):
                sl = slice(c * CHUNK, (c + 1) * CHUNK)
                pn = psh.tile([128, CHUNK], dt.float32, tag="ph")
                mms = [("wn1a_hi", xt[:, sl]), ("wn1c_hi", ft[:, sl])]
                if W_LO:
                    mms += [("wn1a_lo", xt[:, sl]), ("wn1c_lo", ft[:, sl])]
                for i, (wn, rhs) in enumerate(mms):
                    nc.tensor.matmul(pn[:], wt[wn][:], rhs,
                                     start=(i == 0), stop=False)
                nc.tensor.matmul(pn[:], wn1b_t[:], aggT[:, sl],
                                 start=False, stop=True)
                n_hi = wp.tile([128, CHUNK], dt.float16, tag="eh")
                nc.scalar.activation(n_hi[:], pn[:], RELU, bias=bn1_t[:, 0:1])
                px = pse.tile([128, 256], dt.float32, tag="pe")
                for i in range(4):
                    lhsT = n_hi[:, 128 * i : 128 * (i + 1)]
                    nc.tensor.matmul(px[:, 64 * i : 64 * i + 64], lhsT,
                                     wt["wn2_hi"][:], start=True, stop=False)
                    nc.tensor.matmul(px[:, 64 * i : 64 * i + 64], lhsT,
                                     wt["wn2_lo"][:], start=False, stop=True)
                x32 = wp.tile([128, 256], dt.float32, tag="eo32")
                nc.vector.tensor_tensor(x32[:], px[:], bn2_t[:],
                                        mybir.AluOpType.add)
                nc.sync.dma_start(
                    o_x[c * CHUNK : (c + 1) * CHUNK, :]
                    .rearrange("(t p) f -> p t f", p=128),
                    x32[:].rearrange("p (t f) -> p t f", t=4),
                )
    nc.finalize()
    return nc


def prepare(x, edge_index, edge_attr, f, We1, be1, We2, be2, Wn1, bn1, Wn2, bn2):
    x = np.asarray(x, np.float32)
    edge_attr = np.asarray(edge_attr, np.float32)
    f = np.asarray(f, np.float32)
    We1 = np.asarray(We1, np.float32); be1 = np.asarray(be1, np.float32)
    We2 = np.asarray(We2, np.float32); be2 = np.asarray(be2, np.float32)
    Wn1 = np.asarray(Wn1, np.float32); bn1 = np.asarray(bn1, np.float32)
    Wn2 = np.asarray(Wn2, np.float32); bn2 = np.asarray(bn2, np.float32)

    plan = _make_plan(edge_index)
    NT = plan["NT"]
    E_CORE = plan["E_CORE"]
    nb = plan["nb"]
    NSLAB = E_CORE // GCH
    has_be2 = bool(np.any(be2))

    key = (NT, tuple(int(t) for t in plan["T_w"]), has_be2, W_LO, AGG_LO)
    if key not in _CACHE:
        _CACHE[key] = _build_bass(NT, plan["win_t0"], plan["win_of_t"], has_be2)
    nc = _CACHE[key]

    # ---- replicated arrays ----
    xhl = np.zeros((N_NODES + 1, 128), np.float16)
    xhl[:N_NODES] = _hl_cat(x)
    ea_hl_full = _hl_cat(edge_attr)
    f_hl_full = _hl_cat(f)
    x_hl_full = xhl[:N_NODES]

    w1a_hi, w1a_lo = _dup(We1[0:64])
    w1b_hi16, w1b_lo16 = _hl(We1[64:128])      # [64,128] fp16
    w1bd_hi = np.concatenate([w1b_hi16, w1b_hi16], axis=0)
    w1bd_lo = np.concatenate([w1b_lo16, np.zeros_like(w1b_lo16)], axis=0)
    w1c_hi, w1c_lo = _dup(We1[128:192])
    we2_hi, we2_lo = _hl(We2)
    wn1a_hi, wn1a_lo = _dup(Wn1[0:64])
    wn1c_hi, wn1c_lo = _dup(Wn1[128:192])
    wn1b = Wn1[64:128].copy()
    wn2_hi, wn2_lo = _hl(Wn2)

    iota_rep = np.tile(np.arange(128, dtype=np.float16), (128, 1))
    iotac = np.arange(128, dtype=np.float32).reshape(128, 1)
    ones1 = np.ones((1, 128), np.float16)
    ident = np.eye(128, dtype=np.float32)
    b1c = be1.reshape(128, 1)
    b2r = np.tile(be2.reshape(1, 64), (128, 4))
    bn1c = bn1.reshape(128, 1)
    bn2r = np.tile(bn2.reshape(1, 64), (128, 4))

    rep = dict(
        xhl=xhl, iotar=iota_rep, iotac=iotac, ones1=ones1, ident=ident,
        w1a_hi=w1a_hi, w1a_lo=w1a_lo, w1bd_hi=w1bd_hi, w1bd_lo=w1bd_lo,
        w1c_hi=w1c_hi, w1c_lo=w1c_lo, we2_hi=we2_hi, we2_lo=we2_lo,
        wn1a_hi=wn1a_hi, wn1a_lo=wn1a_lo, wn1c_hi=wn1c_hi, wn1c_lo=wn1c_lo,
        wn2_hi=wn2_hi, wn2_lo=wn2_lo, wn1b=wn1b,
        b1c=b1c, b2r=b2r, bn1c=bn1c, bn2r=bn2r,
    )

    in_maps = []
    for k in range(NCORES):
        sor = plan["slot_orig"][k]
        m = sor >= 0
        eahl = np.zeros((E_CORE, 128), np.float16)
        eahl[m] = ea_hl_full[sor[m]]
        base = nb[k]
        hi_n = min(base + NC_PAD, N_NODES)
        xn = np.zeros((NC_PAD, 128), np.float16)
        xn[: hi_n - base] = x_hl_full[base:hi_n]
        fn = np.zeros((NC_PAD, 128), np.float16)
        fn[: hi_n - base] = f_hl_full[base:hi_n]
        dwin16 = plan["slot_dwin"][k].astype(np.float16)
        im = dict(rep)
        im.update(
            sidx=_wrap_idx(plan["slot_src"][k]),
            eahl=eahl,
            dwin=dwin16.reshape(NT, 128).T.copy(),
            dwf=dwin16.reshape(NSLAB, GCH).copy(),
            xn=xn, fn=fn,
        )
        in_maps.append(im)

    return nc, plan, in_maps


def kernel(**inputs):
    nc, plan, in_maps = prepare(**inputs)
    nb = plan["nb"]
    res = run_bass_kernel_spmd(nc, in_maps, core_ids=list(range(NCORES)),
                               trace=TRACE)
    global LAST_RESULT
    LAST_RESULT = res

    edge_out = np.empty((N_EDGES, D), np.float32)
    x_out = np.empty((N_NODES, D), np.float32)
    for k in range(NCORES):
        sor = plan["slot_orig"][k]
        m = sor >= 0
        edge_out[sor[m]] = res.results[k]["o_eo"][m]
        x_out[nb[k] : nb[k + 1]] = res.results[k]["o_x"][: nb[k + 1] - nb[k]]
    return (x_out, edge_out)
